# revision 41
# baseline (speedup 1.0000x reference)
"""Trainium2 Bass kernel for nn_CrossAttention (B=4, Q=512, KV=2048, H=16 heads,
HID=1024, dh=64), sharded over 8 NeuronCores: data-parallel over batch (4) x
tensor-parallel over heads (2 groups of 8 heads).

Core c = 2*b + g handles batch b, head-group g (hidden slice g*512..g*512+512).

All operands are pre-transposed on the HOST (numpy) so the device program has
zero PE transposes:
  kvT_in [HID, KV], qT_in [HID, Q], wkT/wvT/wqT = W_g.T [HID, HDS],
  woT = Wo[:, g].T [HDS, HID].

Per-core program (matmuls in fp32r, PSUM fp32), pipelined over four 512-row
kv quarters with attention lagging projections by one quarter so ScalarE exp
overlaps TensorE projection work:
  - q-proj: qT[hd, q] = wqT.T @ qT_in                 (hd on partitions)
  - per quarter c: kT_c = wkT.T @ kvT_c ; vA_c = kvT_c.T @ wvT (+ ones col)
  - attention (quarter c, after proj of c+1): per head h, per kv-block:
    scoresT = kT.T @ qT (K=64), probs = exp(scale*s + maskbias) on ScalarE,
    avps(+sums row) = vA.T @ probs accumulated over the quarter in PSUM,
    then one DVE add into the per-head SBUF accumulator attnAcc[h].
  - normalize: attnT = attnAcc[0:64] * (1/attnAcc[64]) via a K=1 broadcast
    matmul and a DVE multiply straight out of PSUM
  - out-proj + bias (bias preloaded on g==0 cores only)
  - pairwise ReduceScatter(add) over q rows -> each core returns 256 q rows

kernel(**inputs) takes full inputs, transposes/shards on host, runs SPMD on
cores 0-7, and reassembles the (4, 512, 1024) output.
"""

import numpy as np

import concourse.bass as bass
import concourse.mybir as mybir
import concourse.tile as tile
from concourse import bacc
from concourse.bass_utils import run_bass_kernel_spmd

N_CORES = 8
P = 128
B, Q, KV, HID = 4, 512, 2048, 1024
HDS = 512          # head-dim slice per core (8 heads x 64)
NHEADS = 8         # heads per core
DH = 64
NQTR = 4           # kv quarters of 512 rows
QTR = KV // NQTR   # 512
SCALE = 0.125      # 1/sqrt(64)
MASK_BIG = 1e30
KB = HID // P      # 8 contraction blocks
MB = HDS // P      # 4 head-dim blocks (2 heads each)

F32 = mybir.dt.float32
F32R = mybir.dt.float32r


def _build(loop_k: int = 0, use_f32r: bool = True, analysis: bool = False):
    """Build the SPMD program. loop_k>0 wraps the compute in a For_i hardware
    loop (for timing); loop_k=-1 reads the trip count from input "k_in" at
    runtime (one NEFF for any k). The collective + final DMAs stay outside
    the loop. analysis=True builds a 1-core, collective-free variant for
    TimelineSim."""
    OP_DT = F32R if use_f32r else F32

    nc = bacc.Bacc("TRN2", target_bir_lowering=False, debug=False,
                   num_devices=1 if analysis else N_CORES)

    k_in = (nc.dram_tensor("k_in", [1, 1], mybir.dt.uint32,
                           kind="ExternalInput")
            if loop_k == -1 else None)

    qT_in = nc.dram_tensor("qT_in", [HID, Q], OP_DT, kind="ExternalInput")
    kvT_in = nc.dram_tensor("kvT_in", [HID, KV], OP_DT, kind="ExternalInput")
    wqT_in = nc.dram_tensor("wqT", [HID, HDS], OP_DT, kind="ExternalInput")
    wkT_in = nc.dram_tensor("wkT", [HID, HDS], OP_DT, kind="ExternalInput")
    wvT_in = nc.dram_tensor("wvT", [HID, HDS], OP_DT, kind="ExternalInput")
    woT_in = nc.dram_tensor("woT", [HDS, HID], OP_DT, kind="ExternalInput")
    bo_in = nc.dram_tensor("bo", [1, HID], OP_DT, kind="ExternalInput")
    mask_in = nc.dram_tensor("mask_f", [KV], F32, kind="ExternalInput")
    out_ext = nc.dram_tensor("out", [Q // 2, HID], F32, kind="ExternalOutput")

    cc_in = nc.dram_tensor("cc_in", [Q, HID], F32)
    cc_out = nc.dram_tensor("cc_out", [Q // 2, HID], F32)

    with tile.TileContext(nc) as tc:
        with (
            tc.tile_pool(name="persist", bufs=1) as pp,
            tc.tile_pool(name="weights", bufs=1) as wp,
            tc.tile_pool(name="kvt", bufs=2) as kvp,
            tc.tile_pool(name="kva", bufs=2) as kap,
            tc.tile_pool(name="probs", bufs=4) as probp,
            tc.tile_pool(name="small", bufs=2) as smallp,
            tc.tile_pool(name="out", bufs=4) as outp,
            tc.tile_pool(name="psum_p", bufs=3, space="PSUM") as psp,
            tc.tile_pool(name="psum_s", bufs=3, space="PSUM") as pss,
            tc.tile_pool(name="psum_av", bufs=2, space="PSUM") as psav,
        ):
            # ---- static setup (outside any timing loop) ----
            ones1_f = pp.tile([1, P], F32)
            nc.vector.memset(ones1_f[:], 1.0)
            ones1 = pp.tile([1, P], OP_DT)
            nc.vector.tensor_copy(out=ones1[:], in_=ones1_f[:])
            ones8_f = pp.tile([P, NHEADS], F32)
            nc.vector.memset(ones8_f[:], 1.0)
            ones8 = pp.tile([P, NHEADS], OP_DT)
            nc.vector.tensor_copy(out=ones8[:], in_=ones8_f[:])

            def body():
                # ---- weight / small DMAs ----
                mask_sb = pp.tile([P, KV // P], F32, tag="mask_sb")
                nc.sync.dma_start(
                    mask_sb[:], mask_in.ap().rearrange("(n p) -> p n", p=P)
                )
                bo_r = pp.tile([1, HID], OP_DT, tag="bo_r")
                nc.sync.dma_start(bo_r[:], bo_in[:, :])

                wkT = [wp.tile([P, HDS], OP_DT, tag=f"wkT{kb}", name=f"wkT{kb}")
                       for kb in range(KB)]
                wvT = [wp.tile([P, HDS], OP_DT, tag=f"wvT{kb}", name=f"wvT{kb}")
                       for kb in range(KB)]
                for kb in range(KB):
                    nc.sync.dma_start(wkT[kb][:], wkT_in[kb * P:(kb + 1) * P, :])

                # kvT quarter tiles: kvq[c][kb] = [P, QTR]
                def load_kv_quarter(c):
                    tiles = []
                    for kb in range(KB):
                        t = kvp.tile([P, QTR], OP_DT, tag=f"kvq{kb}",
                                     name=f"kvq{c}_{kb}")
                        nc.sync.dma_start(
                            t[:], kvT_in[kb * P:(kb + 1) * P,
                                         c * QTR:(c + 1) * QTR])
                        tiles.append(t)
                    return tiles

                kvq0 = load_kv_quarter(0)

                for kb in range(KB):
                    nc.sync.dma_start(wvT[kb][:], wvT_in[kb * P:(kb + 1) * P, :])

                wqT = [wp.tile([P, HDS], OP_DT, tag=f"wqT{kb}", name=f"wqT{kb}")
                       for kb in range(KB)]
                for kb in range(KB):
                    nc.sync.dma_start(wqT[kb][:], wqT_in[kb * P:(kb + 1) * P, :])
                # staged through the kv-quarter pool slots (dead after q-proj)
                qTi = [kvp.tile([P, Q], OP_DT, tag=f"kvq{kb}", name=f"qTi{kb}")
                       for kb in range(KB)]
                for kb in range(KB):
                    nc.sync.dma_start(qTi[kb][:], qT_in[kb * P:(kb + 1) * P, :])

                kvq1 = load_kv_quarter(1)

                woT = [pp.tile([P, HID], OP_DT, tag=f"woT{cb}", name=f"woT{cb}")
                       for cb in range(MB)]
                for cb in range(MB):
                    nc.sync.dma_start(woT[cb][:], woT_in[cb * P:(cb + 1) * P, :])

                kvq2 = load_kv_quarter(2)
                kvq3 = load_kv_quarter(3)
                kvq = [kvq0, kvq1, kvq2, kvq3]

                # ---- mask bias / output bias broadcast ----
                bias16 = pp.tile([P, KV // P], F32, tag="bias16")
                # (m - 1) * BIG : 0 where mask true, -BIG where false
                nc.vector.tensor_scalar(
                    bias16[:], mask_sb[:], -1.0, MASK_BIG,
                    mybir.AluOpType.add, mybir.AluOpType.mult,
                )
                # per-head attention accumulators in SBUF (rows 0..63 = AV,
                # row 64 = sum of probs)
                attnAcc = [pp.tile([DH + 1, Q], F32, tag=f"attnAcc{h}",
                                   name=f"attnAcc{h}")
                           for h in range(NHEADS)]
                attnT = [pp.tile([P, Q], OP_DT, tag=f"attnT{t}",
                                 name=f"attnT{t}")
                         for t in range(MB)]

                def proj_quarter(c):
                    """k/v projections of quarter c -> (kT_c, vA_c)."""
                    kT_c = [kap.tile([P, QTR], OP_DT, tag=f"kT{mbh}",
                                     name=f"kT{c}_{mbh}")
                            for mbh in range(MB)]
                    for mbh in range(MB):
                        kps = psp.tile([P, 512], F32, tag="proj_ps")
                        for kb in range(KB):
                            nc.tensor.matmul(
                                kps[:],
                                wkT[kb][:, mbh * P:(mbh + 1) * P],
                                kvq[c][kb][:],
                                start=(kb == 0), stop=(kb == KB - 1),
                            )
                        nc.vector.tensor_copy(out=kT_c[mbh][:], in_=kps[:])
                    vA_c = [kap.tile([P, NHEADS * (DH + 1)], OP_DT,
                                     tag=f"vA{mb4}", name=f"vA{c}_{mb4}")
                            for mb4 in range(4)]
                    for mb4 in range(4):
                        vps = psp.tile([P, HDS], F32, tag="proj_ps")
                        for kb in range(KB):
                            nc.tensor.matmul(
                                vps[:],
                                kvq[c][kb][:, mb4 * P:(mb4 + 1) * P],
                                wvT[kb][:],
                                start=(kb == 0), stop=(kb == KB - 1),
                            )
                        dst = vA_c[mb4][:].rearrange("p (h d) -> p h d",
                                                     d=DH + 1)
                        src = vps[:].rearrange("p (h d) -> p h d", d=DH)
                        nc.vector.tensor_copy(out=dst[:, :, 0:DH], in_=src[:])
                        nc.vector.tensor_copy(
                            out=dst[:, :, DH:DH + 1],
                            in_=ones8[:].rearrange("p (h o) -> p h o", o=1),
                        )
                    return kT_c, vA_c

                def attn_quarter(c, kT_c, vA_c):
                    """scores+exp+AV for all heads over quarter c; folds into
                    attnAcc (copy on c==0, add after)."""
                    for h in range(NHEADS):
                        mb = h // 2
                        off = (h % 2) * DH
                        avps = psav.tile([DH + 1, Q], F32, tag="av_ps")
                        for kvb in range(4):
                            sps = pss.tile([P, Q], F32, tag="s_ps")
                            nc.tensor.matmul(
                                sps[:],
                                kT_c[mb][off:off + DH,
                                         kvb * P:(kvb + 1) * P],
                                qT[mb][off:off + DH, :],
                                start=True, stop=True,
                            )
                            probs = probp.tile([P, Q], OP_DT, tag="probs")
                            nc.scalar.activation(
                                probs[:], sps[:],
                                mybir.ActivationFunctionType.Exp,
                                bias=bias16[:, c * 4 + kvb:c * 4 + kvb + 1],
                                scale=SCALE,
                            )
                            nc.tensor.matmul(
                                avps[:],
                                vA_c[kvb][:, h * (DH + 1):(h + 1) * (DH + 1)],
                                probs[:],
                                start=(kvb == 0), stop=(kvb == 3),
                            )
                        if c == 0:
                            nc.vector.tensor_copy(out=attnAcc[h][:],
                                                  in_=avps[:])
                        else:
                            nc.vector.tensor_tensor(
                                attnAcc[h][:], avps[:], attnAcc[h][:],
                                mybir.AluOpType.add,
                            )
                        if c == NQTR - 1 and h > 0:
                            normalize_head(h - 1)
                            if h % 2 == 0:
                                # attnT[(h-1)//2] complete: fold into the
                                # open wave-1 out-proj PSUM groups
                                outproj_partial((h - 1) // 2)
                    if c == NQTR - 1:
                        normalize_head(NHEADS - 1)
                        outproj_partial(MB - 1)

                def normalize_head(h):
                    """attnT rows for head h = attnAcc[h][0:64] / sums row.
                    Broadcast runs on GpSimd so PE stays out of the chain."""
                    mb = h // 2
                    off = (h % 2) * DH
                    recip = smallp.tile([1, Q], OP_DT, tag="recip")
                    with nc.allow_low_precision(reason="f32r recip operand"):
                        nc.vector.reciprocal(recip[:],
                                             attnAcc[h][DH:DH + 1, :])
                    rbc = smallp.tile([DH, Q], OP_DT, tag="rbc")
                    nc.gpsimd.partition_broadcast(rbc[:], recip[:])
                    nc.vector.tensor_tensor(
                        attnT[mb][off:off + DH, :],
                        attnAcc[h][0:DH, :], rbc[:],
                        mybir.AluOpType.mult,
                    )

                # out-proj groups: hdb partials accumulate in PSUM; the bias
                # lands as a K=1 ones-column matmul; DMA reads PSUM directly.
                # Wave-1 groups start while the last attention quarter runs.
                WAVE1 = [(0, 0), (0, 1), (1, 0)]
                wave1 = {}

                def outproj_group_mm(ops, qb, ob, mb):
                    nc.tensor.matmul(
                        ops[:],
                        attnT[mb][:, qb * P:(qb + 1) * P],
                        woT[mb][:, ob * 512:(ob + 1) * 512],
                        start=(mb == 0), stop=False,
                    )
                    if mb == MB - 1:
                        nc.tensor.matmul(
                            ops[:], ones1[:, :P],
                            bo_r[:, ob * 512:(ob + 1) * 512],
                            start=False, stop=True,
                        )
                        oc = outp.tile([P, 512], F32, tag="out_chunk",
                                       name=f"oc_{qb}_{ob}")
                        nc.any.tensor_copy(out=oc[:], in_=ops[:])
                        nc.sync.dma_start(
                            cc_in[qb * P:(qb + 1) * P,
                                  ob * 512:(ob + 1) * 512],
                            oc[:],
                        )

                def outproj_partial(mb):
                    for qb, ob in WAVE1:
                        if mb == 0:
                            wave1[(qb, ob)] = psp.tile(
                                [P, 512], F32, tag="proj_ps",
                                name=f"w1_{qb}_{ob}")
                        outproj_group_mm(wave1[(qb, ob)], qb, ob, mb)

                # ---- q-proj + quarter pipeline (zero skew: the scheduler
                # fills exp-wait gaps in attn(c) with proj(c+1) matmuls) ----
                kv_proj = [proj_quarter(0)]

                qT = [pp.tile([P, Q], OP_DT, tag=f"qT{mb}", name=f"qT{mb}")
                      for mb in range(MB)]
                for mb in range(MB):
                    qps = psp.tile([P, 512], F32, tag="proj_ps")
                    for kb in range(KB):
                        nc.tensor.matmul(
                            qps[:],
                            wqT[kb][:, mb * P:(mb + 1) * P],
                            qTi[kb][:],
                            start=(kb == 0), stop=(kb == KB - 1),
                        )
                    nc.vector.tensor_copy(out=qT[mb][:], in_=qps[:])

                for c in range(NQTR):
                    attn_quarter(c, *kv_proj.pop())
                    if c + 1 < NQTR:
                        kv_proj.append(proj_quarter(c + 1))

                # ---- out projection wave 2 (groups not started in-quarter) --
                for qb in range(Q // P):
                    for ob in range(2):
                        if (qb, ob) in WAVE1:
                            continue
                        ops = psp.tile([P, 512], F32, tag="proj_ps")
                        for hdb in range(MB):
                            outproj_group_mm(ops, qb, ob, hdb)

            if loop_k == -1:
                kt = pp.tile([1, 1], mybir.dt.uint32)
                nc.sync.dma_start(kt[:], k_in[:, :])
                regs = []
                for eng in (nc.tensor, nc.vector, nc.scalar, nc.gpsimd,
                            nc.sync):
                    r = eng.alloc_register(f"kreg_{eng.engine}")
                    eng.reg_load(r, kt[:])
                    regs.append(r)
                kval = nc.snap(bass.RegisterHandles(regs),
                               min_val=1, max_val=65536)
                import os
                stag = bool(int(os.environ.get("KBENCH_STAGGER", "0")))
                with tc.For_i(0, kval, staggered_reset=stag):
                    body()
            elif loop_k > 0:
                with tc.For_i(0, loop_k):
                    body()
            else:
                body()

            # ---- pairwise reduce-scatter over q rows ----
            if analysis:
                nc.sync.dma_start(out_ext[:, :], cc_in[: Q // 2, :])
            else:
                nc.gpsimd.collective_compute(
                    "ReduceScatter",
                    mybir.AluOpType.add,
                    replica_groups=[[0, 1], [2, 3], [4, 5], [6, 7]],
                    ins=[cc_in.ap().opt()],
                    outs=[cc_out.ap().opt()],
                )
                nc.sync.dma_start(out_ext[:, :], cc_out[:, :])

    nc.compile()
    return nc


_CACHE = {}


def _get_nc(loop_k: int = 0, use_f32r: bool = True):
    key = (loop_k, use_f32r)
    if key not in _CACHE:
        _CACHE[key] = _build(loop_k, use_f32r)
    return _CACHE[key]


def make_in_maps(query, key_value, mask, Wq, Wk, Wv, Wo, bo):
    query = np.asarray(query, dtype=np.float32)
    key_value = np.asarray(key_value, dtype=np.float32)
    mask_f = np.asarray(mask).astype(np.float32)
    Wq = np.asarray(Wq, dtype=np.float32)
    Wk = np.asarray(Wk, dtype=np.float32)
    Wv = np.asarray(Wv, dtype=np.float32)
    Wo = np.asarray(Wo, dtype=np.float32)
    bo = np.asarray(bo, dtype=np.float32).reshape(1, HID)
    zero_bo = np.zeros_like(bo)

    # host-side transposes, shared across the cores that use them
    qT = [np.ascontiguousarray(query[b].T) for b in range(B)]
    kvT = [np.ascontiguousarray(key_value[b].T) for b in range(B)]
    wqT = [np.ascontiguousarray(Wq[g * HDS:(g + 1) * HDS, :].T)
           for g in range(2)]
    wkT = [np.ascontiguousarray(Wk[g * HDS:(g + 1) * HDS, :].T)
           for g in range(2)]
    wvT = [np.ascontiguousarray(Wv[g * HDS:(g + 1) * HDS, :].T)
           for g in range(2)]
    woT = [np.ascontiguousarray(Wo[:, g * HDS:(g + 1) * HDS].T)
           for g in range(2)]

    in_maps = []
    for c in range(N_CORES):
        b, g = c // 2, c % 2
        in_maps.append({
            "qT_in": qT[b],
            "kvT_in": kvT[b],
            "wqT": wqT[g],
            "wkT": wkT[g],
            "wvT": wvT[g],
            "woT": woT[g],
            "bo": bo if g == 0 else zero_bo,
            "mask_f": np.ascontiguousarray(mask_f[b]),
        })
    return in_maps


def kernel(query, key_value, mask, Wq, Wk, Wv, Wo, bo):
    nc = _get_nc(0, True)
    in_maps = make_in_maps(query, key_value, mask, Wq, Wk, Wv, Wo, bo)
    res = run_bass_kernel_spmd(nc, in_maps, list(range(N_CORES))).results
    out = np.empty((B, Q, HID), dtype=np.float32)
    for b_i in range(B):
        out[b_i, : Q // 2] = res[2 * b_i]["out"]
        out[b_i, Q // 2:] = res[2 * b_i + 1]["out"]
    return out
